# revision 29
# baseline (speedup 1.0000x reference)
"""Trainium2 Bass kernel for a pre-norm transformer block (nn_Block_25752623907165).

Sharding: data-parallel over batch B=8 across the 8 NeuronCores (one batch
element per core, zero collectives).

v2 restructure vs baseline:
  - LN gains/biases folded into weights/biases on the host (qkv/fc1 weights
    absorb g; qkv/fc1 biases absorb b; the v-branch bias is folded into the
    proj bias since softmax rows sum to 1).
  - Attention inner loop software-pipelined: AV matmuls are emitted one unit
    behind their scores matmuls, with next-pair QK GEMM chunks (and later the
    invD-broadcast/normalize work) interleaved as PE filler, so the PE never
    idles on the ACT exp and the HAM clock gate stays at 2.4 GHz.
  - Chunked xT DMA + warm-up matmuls kill the cold start; proj is interleaved
    with LN2 stats; fc1's first m-group is pipelined against LN2 output chunks.
  - LayerNorm 2nd pass in bf16 (2x DVE rate); rstd = exp(-0.5*ln(var)) keeps
    ACT on the natural_log_exp table set through attention (2 table loads
    total); residual+bias fused in one scalar_tensor_tensor op.
"""

import numpy as np
import ml_dtypes
from collections import deque

EMBED = 1024
HEADS = 16
HEAD_DIM = 64
HIDDEN = 4096
N_TOK = 1024
B = 8
N_CORES = 8
EPS = 1e-5
P = 128
CSUB = EMBED // P          # 8
HSUB = HIDDEN // P         # 32
QW = 512                   # PSUM bank = 512 fp32

BF16 = ml_dtypes.bfloat16
FP8 = ml_dtypes.float8_e4m3
FP8_MAX = 224.0

_CACHE = {}


def _fp8_lhsT_chunks(w, n_mtiles):
    """[K, M] fp32 -> per-channel-scaled fp8 lhsT chunks + invscale percol.

    Returns (chunks [n_mtiles, 128, K//128, 128] fp8, invsc [128, n_mtiles] f32)
    """
    K, M = w.shape
    sc = FP8_MAX / np.abs(w).max(axis=0)          # [M]
    wq = np.clip(w * sc[None, :], -240., 240.).astype(FP8)
    a = wq.reshape(K // P, P, n_mtiles, P).transpose(2, 1, 0, 3)
    return np.ascontiguousarray(a), _pack_percol(1.0 / sc)


# ---------------------------------------------------------------------------
# host-side packing helpers
# ---------------------------------------------------------------------------

def _pack_lhsT_chunks(w, n_mtiles):
    """[K, M] fp32 -> [n_mtiles, 128, K//128, 128] bf16 (lhsT tiles for PE)."""
    K, M = w.shape
    a = w.reshape(K // P, P, n_mtiles, P).transpose(2, 1, 0, 3)
    return np.ascontiguousarray(a.astype(BF16))


def _pack_rhs(w):
    """[K, M] fp32 -> [128, K//128, M] bf16 (moving-operand layout)."""
    K, M = w.shape
    a = w.reshape(K // P, P, M).transpose(1, 0, 2)
    return np.ascontiguousarray(a.astype(BF16))


def _pack_percol(v):
    """[F] fp32 -> [128, F//128] fp32: column m holds features m*128..m*128+127."""
    F = v.shape[0]
    return np.ascontiguousarray(v.reshape(F // P, P).T.astype(np.float32))


def _pack_xT(xb):
    """[N, C] fp32 -> [128, C//128, N] fp32 (transposed, partition-major)."""
    xT = xb.T  # [C, N]
    a = xT.reshape(CSUB, P, N_TOK).transpose(1, 0, 2)
    return np.ascontiguousarray(a.astype(BF16))


def _unpack_yT(yT):
    """[128, C//128, N] fp32 -> [N, C] fp32."""
    full = yT.transpose(1, 0, 2).reshape(EMBED, N_TOK).astype(np.float32)
    return np.ascontiguousarray(full.T)


# ---------------------------------------------------------------------------
# kernel build
# ---------------------------------------------------------------------------

WV_SCALE = 64.0


def _build():
    import concourse.bacc as bacc
    import concourse.mybir as mybir
    import concourse.tile as tile
    from contextlib import ExitStack

    dt = mybir.dt
    AF = mybir.ActivationFunctionType
    OP = mybir.AluOpType

    nc = bacc.Bacc("TRN2", target_bir_lowering=False, debug=False)

    f32, bf16, fp8 = dt.float32, dt.bfloat16, dt.float8e4

    def dram(name, shape, d=f32, out=False):
        return nc.dram_tensor(name, list(shape), d,
                              kind="ExternalOutput" if out else "ExternalInput").ap()

    xT_d = dram("xT", [P, CSUB, N_TOK], bf16)
    wqk_d = dram("wqk", [16, P, CSUB, P], fp8)      # lhsT chunks, q|k features
    wv_d = dram("wv", [P, CSUB, EMBED], fp8)        # rhs layout
    bqk_d = dram("bqk", [P, 16])
    wpr_d = dram("wpr", [CSUB, P, CSUB, P], fp8)
    bpr_d = dram("bpr", [P, CSUB])
    wf1_d = dram("wf1", [HSUB, P, CSUB, P], fp8)
    bf1_d = dram("bf1", [P, HSUB])
    wf2_d = dram("wf2", [CSUB, P, HSUB, P], fp8)
    bf2_d = dram("bf2", [P, CSUB])
    iqk_d = dram("iqk", [P, 16])
    ipr_d = dram("ipr", [P, CSUB])
    if1_d = dram("if1", [P, HSUB])
    if2_d = dram("if2", [P, CSUB])
    yT_d = dram("yT", [P, CSUB, N_TOK], bf16, out=True)

    with tile.TileContext(nc) as tc, ExitStack() as ctx:
        const = ctx.enter_context(tc.tile_pool(name="const", bufs=1))
        persist = ctx.enter_context(tc.tile_pool(name="persist", bufs=1))
        smalls = ctx.enter_context(tc.tile_pool(name="smalls", bufs=1))
        w8 = ctx.enter_context(tc.tile_pool(name="w8", bufs=5))

        # ---- constants ---------------------------------------------------
        ones_sq = const.tile([P, P], bf16)      # 1/1024 : layernorm mean lhsT
        nc.vector.memset(ones_sq[:], 1.0 / EMBED)
        ones_bc = const.tile([P, P], bf16)      # 1.0 : K=1 broadcast lhsT rows
        nc.vector.memset(ones_bc[:], 1.0)
        warm_rhs = const.tile([P, QW], bf16)
        nc.vector.memset(warm_rhs[:], 0.001)
        dexp = const.tile([P, QW], bf16)
        nc.scalar.activation(dexp[:], warm_rhs[:], AF.Exp)

        bqk_sb = const.tile([P, 16], f32)
        nc.sync.dma_start(bqk_sb[:], bqk_d[:])
        bpr_sb = const.tile([P, CSUB], f32)
        nc.sync.dma_start(bpr_sb[:], bpr_d[:])
        bf1_sb = const.tile([P, HSUB], f32)
        nc.sync.dma_start(bf1_sb[:], bf1_d[:])
        bf2_sb = const.tile([P, CSUB], f32)
        nc.sync.dma_start(bf2_sb[:], bf2_d[:])
        iqk_sb = const.tile([P, 16], f32)
        nc.sync.dma_start(iqk_sb[:], iqk_d[:])
        ipr_sb = const.tile([P, CSUB], f32)
        nc.sync.dma_start(ipr_sb[:], ipr_d[:])
        if1_sb = const.tile([P, HSUB], f32)
        nc.sync.dma_start(if1_sb[:], if1_d[:])
        if2_sb = const.tile([P, CSUB], f32)
        nc.sync.dma_start(if2_sb[:], if2_d[:])
        DR = mybir.MatmulPerfMode.DoubleRow

        # ---- persistent activations -------------------------------------
        xT = persist.tile([P, CSUB, N_TOK], bf16)

        # LN stat broadcast tiles (rotate bufs=1 between LN1/LN2)
        def ln_tiles(which):
            mu_bf = smalls.tile([P, N_TOK], bf16, tag="mu", name=f"mu{which}")
            rstd_bf = smalls.tile([P, N_TOK], bf16, tag="rstd",
                                  name=f"rstd{which}")
            mu2 = smalls.tile([P, N_TOK], bf16, tag="mu2", name=f"mu2{which}")
            var_t = smalls.tile([P, N_TOK], bf16, tag="var", name=f"var{which}")
            return mu_bf, rstd_bf, mu2, var_t

        def emit_ln_evict(mu_ps, sq_ps, mu_bf, rstd_bf, mu2, var_t, which):
            # mu broadcast (bf16); u = E[x^2]+EPS-mu^2-1 (|u|<~0.15);
            # ln(1+u) ~= u + u^2*(u/3 - 1/2) on DVE (keeps ACT on the exp
            # table set: no Ln -> no table-set ping-pong on the critical path)
            nc.scalar.activation(mu_bf[:], mu_ps[:], AF.Copy)
            nc.vector.tensor_tensor(mu2[:], mu_bf[:], mu_bf[:], OP.mult)
            nc.vector.scalar_tensor_tensor(var_t[:], sq_ps[:],
                                           float(EPS) - 1.0, mu2[:],
                                           OP.add, OP.subtract)
            u2 = smalls.tile([P, N_TOK], bf16, tag="lnu2", name=f"u2_{which}")
            nc.vector.tensor_tensor(u2[:], var_t[:], var_t[:], OP.mult)
            w = smalls.tile([P, N_TOK], bf16, tag="lnw", name=f"w_{which}")
            nc.vector.tensor_scalar(w[:], var_t[:], 1.0 / 3.0, -0.5,
                                    OP.mult, OP.add)
            qq = smalls.tile([P, N_TOK], bf16, tag="lnq", name=f"q_{which}")
            nc.vector.tensor_tensor(qq[:], u2[:], w[:], OP.mult)
            lnv = smalls.tile([P, N_TOK], bf16, tag="lnv", name=f"lnv_{which}")
            nc.vector.tensor_tensor(lnv[:], var_t[:], qq[:], OP.add)
            nc.scalar.activation(rstd_bf[:], lnv[:], AF.Exp, scale=-0.5)

        # =================================================================
        # Phase 0+1: warmup + chunked x DMA + LN1 stats (pipelined)
        # x (bf16) is staged into hT, then layernormed in place.
        # =================================================================
        oT_cm = tc.tile_pool(name="oT_pool", bufs=1)
        oTp = oT_cm.__enter__()
        oT = oTp.tile([P, CSUB, N_TOK], fp8)

        attn_cm = tc.tile_pool(name="attn_sb", bufs=1)
        attn = attn_cm.__enter__()
        hT = attn.tile([P, CSUB, N_TOK], fp8)
        qkall = attn.tile([P, 16, N_TOK], bf16)
        v65 = attn.tile([P, CSUB, HEADS, 65], bf16)
        nc.vector.memset(v65[:, :, :, 64:65], 1.0)
        OSLOT = 9
        oU = attn.tile([64, OSLOT, N_TOK], bf16)
        DrA = attn.tile([8, N_TOK], f32)
        DrB = attn.tile([4, N_TOK], f32)
        DrC = attn.tile([4, N_TOK], f32)
        invA = attn.tile([8, N_TOK], f32, tag="inv", name="invA")
        invB = attn.tile([4, N_TOK], f32, tag="inv", name="invB")
        invC = attn.tile([4, N_TOK], f32, tag="inv", name="invC")
        invA_bf = attn.tile([8, N_TOK], bf16)
        invB_bf = attn.tile([4, N_TOK], bf16, tag="invBC_bf", name="invB_bf")
        invC_bf = attn.tile([4, N_TOK], bf16, tag="invBC_bf", name="invC_bf")

        wv_cm = tc.tile_pool(name="wv_pool", bufs=1)
        wvp = wv_cm.__enter__()
        wv_sb = wvp.tile([P, CSUB, EMBED], fp8)

        wqk_tiles = {}

        def fetch_wqk(m):
            wt = w8.tile([P, CSUB, P], fp8, tag="w8", name=f"wqk{m}")
            nc.sync.dma_start(wt[:], wqk_d[m])
            wqk_tiles[m] = wt

        ps_stat_cm = tc.tile_pool(name="ps_stat", bufs=1, space="PSUM")
        ps_stat = ps_stat_cm.__enter__()
        mu_ps = ps_stat.tile([P, N_TOK], f32, tag="mu_ps")
        sq_ps = ps_stat.tile([P, N_TOK], f32, tag="sq_ps")
        warm_ps = ps_stat.tile([P, N_TOK], f32, tag="warm")

        def warm(n):
            for _ in range(n):
                nc.tensor.matmul(warm_ps[:, 0:QW], ones_sq[:], warm_rhs[:])

        fetch_wqk(0); fetch_wqk(8)
        warm(10)
        for c in range(CSUB):
            nc.sync.dma_start(xT[:, c, :], xT_d[:, c, :])
            nc.sync.dma_start(wv_sb[:, c, :], wv_d[:, c, :])
            if c == 0:
                fetch_wqk(1); fetch_wqk(9)
            xbc = xT[:, c, :]
            sqc = smalls.tile([P, N_TOK], bf16, tag="sq", bufs=2,
                              name=f"sq{c}")
            nc.vector.tensor_tensor(sqc[:], xbc, xbc, OP.mult)
            for q in range(2):
                sl = slice(q * QW, (q + 1) * QW)
                nc.tensor.matmul(mu_ps[:, sl], ones_sq[:], xbc[:, sl],
                                 start=(c == 0), stop=(c == CSUB - 1))
                nc.tensor.matmul(sq_ps[:, sl], ones_sq[:], sqc[:, sl],
                                 start=(c == 0), stop=(c == CSUB - 1))
            warm(5)

        mu_bf, rstd_bf, mu2, var_t = ln_tiles(1)
        emit_ln_evict(mu_ps, sq_ps, mu_bf, rstd_bf, mu2, var_t, 1)
        warm(26)

        # LN1 2nd pass: bf16 DVE subtract/mult, fp8 cast on ACT -> hT
        for c in range(CSUB):
            t = smalls.tile([P, N_TOK], bf16, tag="lnt", bufs=1,
                            name=f"lnt{c}")
            nc.vector.tensor_tensor(t[:], xT[:, c, :], mu_bf[:], OP.subtract)
            t2 = smalls.tile([P, N_TOK], bf16, tag="lnt2", bufs=2,
                             name=f"lnt2_{c}")
            nc.vector.tensor_tensor(t2[:], t[:], rstd_bf[:], OP.mult)
            nc.scalar.copy(hT[:, c, :], t2[:])

        ps_stat_cm.__exit__(None, None, None)

        # =================================================================
        # Phase 2: qkv GEMM groups (pair-0/1 qk + all v), c-major pipelined
        # =================================================================
        ps_g_cm = tc.tile_pool(name="ps_g", bufs=4, space="PSUM")
        ps_g = ps_g_cm.__enter__()

        def evict_qk(m, ps):
            nc.vector.tensor_scalar(qkall[:, m, :], ps[:],
                                    iqk_sb[:, m:m + 1], bqk_sb[:, m:m + 1],
                                    OP.mult, OP.add)

        def evict_v(mt, ps):
            nc.vector.tensor_scalar(
                v65[:, mt, :, 0:64],
                ps[:].rearrange("p (h d) -> p h d", d=64),
                1.0 / WV_SCALE, None, OP.mult)

        # group entries: ("qk", m) or ("v", mt)
        groups = [[("qk", 0), ("qk", 8), ("v", 0), ("v", 1)],
                  [("v", 2), ("v", 3), ("v", 4), ("v", 5)],
                  [("v", 6), ("v", 7), ("qk", 1), ("qk", 9)]]
        for gi, grp in enumerate(groups):
            pss = []
            for kind, m in grp:
                pss.append(ps_g.tile([P, N_TOK], f32, tag="g",
                                     name=f"g{gi}_{kind}{m}"))
            for c in range(0, CSUB, 2):
                for (kind, m), ps in zip(grp, pss):
                    for q in range(2):
                        sl = slice(q * QW, (q + 1) * QW)
                        if kind == "qk":
                            nc.tensor.matmul(ps[:, sl],
                                             wqk_tiles[m][:, c:c + 2, :],
                                             hT[:, c:c + 2, sl],
                                             start=(c == 0),
                                             stop=(c == CSUB - 2),
                                             perf_mode=DR)
                        else:
                            nc.tensor.matmul(ps[:, sl],
                                             hT[:, c:c + 2,
                                                m * P:(m + 1) * P],
                                             wv_sb[:, c:c + 2, sl],
                                             start=(c == 0),
                                             stop=(c == CSUB - 2),
                                             perf_mode=DR)
            for (kind, m), ps in zip(grp, pss):
                if kind == "qk":
                    evict_qk(m, ps)
                else:
                    evict_v(m, ps)

        ps_g_cm.__exit__(None, None, None)
        wv_cm.__exit__(None, None, None)

        # =================================================================
        # Phase 3: attention, software-pipelined with qk/normalize filler
        # =================================================================
        ps_at_cm = tc.tile_pool(name="ps_at", bufs=1, space="PSUM")
        ps_at = ps_at_cm.__enter__()
        # tags: s (scores, bufs=2), o (AV accum, bufs=1), q (filler, bufs=1)

        wpr_tiles = {}

        def fetch_wpr(m):
            wt = w8.tile([P, CSUB, P], fp8, tag="w8", name=f"wpr{m}")
            nc.sync.dma_start(wt[:], wpr_d[m])
            wpr_tiles[m] = wt

        filler = deque()

        def step(n=1):
            for _ in range(n):
                if filler:
                    filler.popleft()()

        def make_qk_filler(m):
            """Filler steps computing qk chunk m into qkall (psQ rotation)."""
            state = {}

            def s_cq(c, q):
                def run():
                    if c == 0 and q == 0:
                        state["ps"] = ps_at.tile([P, N_TOK], f32, tag="q",
                                                 bufs=1, name=f"qf{m}")
                    sl = slice(q * QW, (q + 1) * QW)
                    # deliberately NOT DoubleRow: filler density keeps the
                    # PE duty high enough that the HAM stays at K=8/8
                    nc.tensor.matmul(state["ps"][:, sl],
                                     wqk_tiles[m][:, c, :], hT[:, c, sl],
                                     start=(c == 0), stop=(c == CSUB - 1))
                return run

            steps = [s_cq(c, q) for c in range(CSUB) for q in range(2)]
            steps.append(lambda: evict_qk(m, state["ps"]))
            return steps

        iD_tiles = {}

        def load_iD_batch(bi, src, rows):
            iD = attn.tile([P, N_TOK], bf16, tag="iD64", bufs=2,
                           name=f"iDb{bi}")
            for j, r in enumerate(rows):
                nc.sync.dma_start(iD[32 * j:32 * j + 1, :], src[r:r + 1, :])
            iD_tiles[bi] = iD

        def make_junk_step(nm):
            def run():
                jt = ps_at.tile([P, N_TOK], f32, tag="s", bufs=2,
                                name=f"j{nm}")
                nc.tensor.matmul(jt[:, 0:QW], ones_sq[:], warm_rhs[:])
            return run

        def make_norm_filler(h):
            """Filler steps normalizing head h into oT (uses psQ for bc)."""
            bi, j = divmod(h, 4)
            state = {}

            def s_bc():
                bc = ps_at.tile([P, N_TOK], f32, tag="q", bufs=1,
                                name=f"bc{h}")
                r = 32 * j
                for q in range(2):
                    sl = slice(q * QW, (q + 1) * QW)
                    nc.tensor.matmul(bc[0:64, sl], ones_bc[r:r + 1, 0:64],
                                     iD_tiles[bi][r:r + 1, sl],
                                     tile_position=(r, 0))
                state["bc"] = bc

            def s_mul():
                if h % 2 == 0:
                    nc.vector.tensor_tensor(oT[0:64, h // 2, :],
                                            oU[:, h % OSLOT, :],
                                            state["bc"][0:64, :], OP.mult)
                else:
                    to = attn.tile([64, N_TOK], fp8, tag="to", bufs=1,
                                   name=f"to{h}")
                    nc.vector.tensor_tensor(to[:], oU[:, h % OSLOT, :],
                                            state["bc"][0:64, :], OP.mult)
                    nc.sync.dma_start(oT[64:128, h // 2, :], to[:])

            return [s_bc, s_mul]

        # attention unit machinery
        e_of = {}     # (h,k) -> exp tile
        o_ps_of = {}  # h -> AV psum tile

        def unit_scores(p, h, k):
            rows = slice(0, 64) if (h % 2) == 0 else slice(64, 128)
            s_ps = ps_at.tile([P, N_TOK], f32, tag="s", bufs=2,
                              name=f"s{h}_{k}")
            for q in range(2):
                sl = slice(q * QW, (q + 1) * QW)
                nc.tensor.matmul(s_ps[:, sl],
                                 qkall[rows, 8 + p, k * P:(k + 1) * P],
                                 qkall[rows, p, sl])
            e = attn.tile([P, N_TOK], bf16, tag="e", bufs=2, name=f"e{h}_{k}")
            nc.scalar.activation(e[:], s_ps[:], AF.Exp, scale=0.125)
            e_of[(h, k)] = e

        def unit_av(h, k):
            if k == 0:
                o_ps_of[h] = ps_at.tile([P, N_TOK], f32, tag="o", bufs=1,
                                        name=f"o{h}")
            o_ps = o_ps_of[h]
            e = e_of.pop((h, k))
            for q in range(2):
                sl = slice(q * QW, (q + 1) * QW)
                nc.tensor.matmul(o_ps[0:65, sl], v65[:, k, h, :], e[:, sl],
                                 start=(k == 0), stop=(k == CSUB - 1))

        def end_head(h):
            o_ps = o_ps_of.pop(h)
            nc.vector.tensor_copy(oU[:, h % OSLOT, :], o_ps[0:64, :])
            d64 = attn.tile([65, N_TOK], f32, tag="d64", bufs=1,
                            name=f"d64_{h}")
            nc.vector.tensor_copy(d64[64:65, :], o_ps[64:65, :])
            if h < 8:
                dst = DrA[h:h + 1, :]
            elif h < 12:
                dst = DrB[h - 8:h - 7, :]
            else:
                dst = DrC[h - 12:h - 11, :]
            nc.sync.dma_start(dst, d64[64:65, :])
            # denominator batches -> reciprocal + normalize fillers
            if h == 7:
                nc.vector.reciprocal_approx_fast(invA[:], DrA[:])
                nc.vector.tensor_scalar(invA_bf[:], invA[:], 16.0, None, OP.mult)
                load_iD_batch(0, invA_bf, [0, 1, 2, 3])
                load_iD_batch(1, invA_bf, [4, 5, 6, 7])
                for hh in range(0, 8):
                    filler.extend(make_norm_filler(hh))
            elif h == 11:
                nc.vector.reciprocal_approx_fast(invB[:], DrB[:])
                nc.vector.tensor_scalar(invB_bf[:], invB[:], 16.0, None, OP.mult)
                load_iD_batch(2, invB_bf, [0, 1, 2, 3])
                filler.append(make_junk_step("jb0"))
                filler.append(make_junk_step("jb1"))
                for hh in range(8, 12):
                    filler.extend(make_norm_filler(hh))
            elif h == 15:
                nc.vector.reciprocal_approx_fast(invC[:], DrC[:])
                nc.vector.tensor_scalar(invC_bf[:], invC[:], 16.0, None, OP.mult)
                load_iD_batch(3, invC_bf, [0, 1, 2, 3])
                for hh in range(12, 16):
                    filler.extend(make_norm_filler(hh))
                fetch_wpr(0)
                fetch_wpr(1)

        # run units with 1-unit AV lag; qk chunks for pair p+2 as filler
        pend = None
        for p in range(8):
            if p + 2 < 8:
                fetch_wqk(p + 2)
                fetch_wqk(8 + p + 2)
                filler.extend(make_qk_filler(p + 2))
                filler.extend(make_qk_filler(8 + p + 2))
            else:
                for jj in range(16):
                    filler.append(make_junk_step(f"p{p}_{jj}"))
            step(2)
            for h in (2 * p, 2 * p + 1):
                for k in range(CSUB):
                    unit_scores(p, h, k)
                    step()
                    if pend is not None:
                        unit_av(*pend)
                        if pend[1] == CSUB - 1:
                            end_head(pend[0])
                    pend = (h, k)
                    step()
        unit_av(*pend)
        end_head(pend[0])
        for jj in range(10):
            make_junk_step(f"dr{jj}")()
        ji = 0
        while filler:
            filler.popleft()()
            jt = ps_at.tile([P, N_TOK], f32, tag="s", bufs=2,
                            name=f"jnk{ji}")
            nc.tensor.matmul(jt[:, 0:QW], ones_sq[:], warm_rhs[:])
            ji += 1

        ps_at_cm.__exit__(None, None, None)
        attn_cm.__exit__(None, None, None)

        # =================================================================
        # Phase 4: proj + residual + LN2 stats (interleaved per m-chunk)
        # =================================================================
        mlp_cm = tc.tile_pool(name="mlp_sb", bufs=1)
        mlp = mlp_cm.__enter__()
        w32_cm = tc.tile_pool(name="w32", bufs=2)
        w32 = w32_cm.__enter__()
        ln2T = mlp.tile([P, CSUB, N_TOK], fp8)
        geluT = mlp.tile([P, HSUB, N_TOK], fp8)

        ps_pr_cm = tc.tile_pool(name="ps_pr", bufs=1, space="PSUM")
        ps_pr = ps_pr_cm.__enter__()
        mu2_ps = ps_pr.tile([P, N_TOK], f32, tag="mu2_ps")
        sq2_ps = ps_pr.tile([P, N_TOK], f32, tag="sq2_ps")

        for m in range(CSUB):
            if m + 2 < CSUB:
                fetch_wpr(m + 2)
            p_ps = ps_pr.tile([P, N_TOK], f32, tag="p", bufs=2,
                              name=f"p{m}")
            for c in range(0, CSUB, 2):
                for q in range(2):
                    sl = slice(q * QW, (q + 1) * QW)
                    nc.tensor.matmul(p_ps[:, sl],
                                     wpr_tiles[m][:, c:c + 2, :],
                                     oT[:, c:c + 2, sl],
                                     start=(c == 0), stop=(c == CSUB - 2),
                                     perf_mode=DR)
            # unscale on ACT, then xT += t + bias (fused)
            t_pr = mlp.tile([P, N_TOK], bf16, tag="tpr", bufs=2,
                            name=f"tpr{m}")
            nc.scalar.activation(t_pr[:], p_ps[:], AF.Copy,
                                 scale=ipr_sb[:, m:m + 1])
            nc.vector.scalar_tensor_tensor(xT[:, m, :], t_pr[:],
                                           bpr_sb[:, m:m + 1], xT[:, m, :],
                                           OP.add, OP.add)
            # LN2 stats for this chunk (straight from bf16 xT)
            xbc = xT[:, m, :]
            sqc = mlp.tile([P, N_TOK], bf16, tag="sq2", bufs=2,
                           name=f"sqB{m}")
            nc.vector.tensor_tensor(sqc[:], xbc, xbc, OP.mult)
            for q in range(2):
                sl = slice(q * QW, (q + 1) * QW)
                nc.tensor.matmul(mu2_ps[:, sl], ones_sq[:], xbc[:, sl],
                                 start=(m == 0), stop=(m == CSUB - 1))
                nc.tensor.matmul(sq2_ps[:, sl], ones_sq[:], sqc[:, sl],
                                 start=(m == 0), stop=(m == CSUB - 1))

        mu_bf2, rstd_bf2, mu2b, var_t2 = ln_tiles(2)
        emit_ln_evict(mu2_ps, sq2_ps, mu_bf2, rstd_bf2, mu2b, var_t2, 2)

        ps_pr_cm.__exit__(None, None, None)

        # =================================================================
        # Phase 5: LN2 2nd pass + fc1 (m-groups; G0 pipelined per chunk)
        # =================================================================
        ps_mm_cm = tc.tile_pool(name="ps_mm", bufs=4, space="PSUM")
        ps_mm = ps_mm_cm.__enter__()

        def warm_mm(n, nm):
            wps = ps_mm.tile([P, N_TOK], f32, tag="mm", name=f"wm{nm}")
            for _ in range(n):
                nc.tensor.matmul(wps[:, 0:QW], ones_sq[:], warm_rhs[:])

        wf1_tiles = {}

        def fetch_wf1(m):
            wt = w8.tile([P, CSUB, P], fp8, tag="w8", name=f"wf1_{m}")
            nc.sync.dma_start(wt[:], wf1_d[m])
            wf1_tiles[m] = wt

        for m in range(4):
            fetch_wf1(m)
        warm_mm(14, "a")

        # LN2 2nd pass chunks: bf16 DVE ops, fp8 cast on ACT -> ln2T
        for c in range(CSUB):
            t = mlp.tile([P, N_TOK], bf16, tag="lnt2", bufs=2,
                         name=f"lnB{c}")
            nc.vector.tensor_tensor(t[:], xT[:, c, :], mu_bf2[:],
                                    OP.subtract)
            t2 = mlp.tile([P, N_TOK], bf16, tag="lnt2b", bufs=2,
                          name=f"lnB2_{c}")
            nc.vector.tensor_tensor(t2[:], t[:], rstd_bf2[:], OP.mult)
            nc.scalar.copy(ln2T[:, c, :], t2[:])

        # G0: m-tiles 0..3, c-major (chases LN2 chunk production)
        g0 = [ps_mm.tile([P, N_TOK], f32, tag="mm", name=f"f1g0_{m}")
              for m in range(4)]
        for c in range(0, CSUB, 2):
            for m in range(4):
                for q in range(2):
                    sl = slice(q * QW, (q + 1) * QW)
                    nc.tensor.matmul(g0[m][:, sl],
                                     wf1_tiles[m][:, c:c + 2, :],
                                     ln2T[:, c:c + 2, sl],
                                     start=(c == 0), stop=(c == CSUB - 2),
                                     perf_mode=DR)
        for m in range(4):
            fetch_wf1(m + 4)
        for m in range(4):
            nc.scalar.activation(geluT[:, m, :], g0[m][:], AF.Gelu,
                                 bias=bf1_sb[:, m:m + 1],
                                 scale=if1_sb[:, m:m + 1])

        # G1..: m-major
        for m in range(4, HSUB):
            if m + 4 < HSUB:
                fetch_wf1(m + 4)
            f_ps = ps_mm.tile([P, N_TOK], f32, tag="mm", name=f"f1_{m}")
            for c in range(0, CSUB, 2):
                for q in range(2):
                    sl = slice(q * QW, (q + 1) * QW)
                    nc.tensor.matmul(f_ps[:, sl],
                                     wf1_tiles[m][:, c:c + 2, :],
                                     ln2T[:, c:c + 2, sl],
                                     start=(c == 0), stop=(c == CSUB - 2),
                                     perf_mode=DR)
            nc.scalar.activation(geluT[:, m, :], f_ps[:], AF.Gelu,
                                 bias=bf1_sb[:, m:m + 1],
                                 scale=if1_sb[:, m:m + 1])

        # =================================================================
        # Phase 6: fc2 + residual -> yT
        # =================================================================
        w2_tiles = {}

        def fetch_wf2(m):
            wt = w32.tile([P, HSUB, P], fp8, tag="w32", name=f"wf2_{m}")
            nc.sync.dma_start(wt[:], wf2_d[m])
            w2_tiles[m] = wt

        fetch_wf2(0)
        for m2 in range(CSUB):
            if m2 + 1 < CSUB:
                fetch_wf2(m2 + 1)
            y_ps = ps_mm.tile([P, N_TOK], f32, tag="mm", name=f"f2_{m2}")
            for k in range(0, HSUB, 2):
                for q in range(2):
                    sl = slice(q * QW, (q + 1) * QW)
                    nc.tensor.matmul(y_ps[:, sl],
                                     w2_tiles[m2][:, k:k + 2, :],
                                     geluT[:, k:k + 2, sl],
                                     start=(k == 0), stop=(k == HSUB - 2),
                                     perf_mode=DR)
            t_f2 = mlp.tile([P, N_TOK], bf16, tag="tpr", bufs=2,
                            name=f"tf2{m2}")
            nc.scalar.activation(t_f2[:], y_ps[:], AF.Copy,
                                 scale=if2_sb[:, m2:m2 + 1])
            nc.vector.scalar_tensor_tensor(xT[:, m2, :], t_f2[:],
                                           bf2_sb[:, m2:m2 + 1], xT[:, m2, :],
                                           OP.add, OP.add)
            nc.sync.dma_start(yT_d[:, m2, :], xT[:, m2, :])

        ps_mm_cm.__exit__(None, None, None)
        w32_cm.__exit__(None, None, None)
        mlp_cm.__exit__(None, None, None)
        oT_cm.__exit__(None, None, None)

    nc.compile()
    return nc


def get_nc():
    if "nc" not in _CACHE:
        _CACHE["nc"] = _build()
    return _CACHE["nc"]


def make_in_maps(x, qkv_w, qkv_b, proj_w, proj_b, fc1_w, fc1_b, fc2_w, fc2_b,
                 ln1_g, ln1_b, ln2_g, ln2_b):
    f = np.float32
    x = np.asarray(x, f)
    qkv_w = np.asarray(qkv_w, f); qkv_b = np.asarray(qkv_b, f)
    proj_w = np.asarray(proj_w, f); proj_b = np.asarray(proj_b, f)
    fc1_w = np.asarray(fc1_w, f); fc1_b = np.asarray(fc1_b, f)
    fc2_w = np.asarray(fc2_w, f); fc2_b = np.asarray(fc2_b, f)
    g1 = np.asarray(ln1_g, f); b1 = np.asarray(ln1_b, f)
    g2 = np.asarray(ln2_g, f); b2 = np.asarray(ln2_b, f)

    # fold LN1 gain/bias into qkv weights/bias
    W1 = qkv_w * g1[:, None]
    b1f = qkv_b + b1 @ qkv_w
    vb = b1f[2 * EMBED:]
    # v-branch bias folds into proj bias (softmax rows sum to 1)
    bprf = proj_b + vb @ proj_w
    # fold LN2 gain/bias into fc1
    W2 = fc1_w * g2[:, None]
    bf1f = fc1_b + b2 @ fc1_w

    wqk_q, iqk = _fp8_lhsT_chunks(W1[:, :2 * EMBED], 16)
    # v: per-tensor scale (uniform weights); proj absorbs the oT x16 scale
    wv_q = np.clip(W1[:, 2 * EMBED:] * WV_SCALE, -240., 240.).astype(FP8)
    Kv, Mv = wv_q.shape
    wv_q = np.ascontiguousarray(
        wv_q.reshape(Kv // P, P, Mv).transpose(1, 0, 2))
    wpr_q, ipr = _fp8_lhsT_chunks(proj_w / 16.0, CSUB)
    wf1_q, if1 = _fp8_lhsT_chunks(W2, HSUB)
    wf2_q, if2 = _fp8_lhsT_chunks(fc2_w, CSUB)
    shared = {
        "wqk": wqk_q, "iqk": iqk,
        "wv": wv_q,
        "bqk": _pack_percol(b1f[:2 * EMBED]),
        "wpr": wpr_q, "ipr": ipr,
        "bpr": _pack_percol(bprf),
        "wf1": wf1_q, "if1": if1,
        "bf1": _pack_percol(bf1f),
        "wf2": wf2_q, "if2": if2,
        "bf2": _pack_percol(fc2_b),
    }
    return [dict(shared, xT=_pack_xT(x[b])) for b in range(B)]


def kernel(**inputs):
    from concourse.bass_utils import run_bass_kernel_spmd

    nc = get_nc()
    in_maps = make_in_maps(**inputs)
    res = run_bass_kernel_spmd(nc, in_maps, core_ids=list(range(N_CORES)))
    out = np.stack([_unpack_yT(res.results[b]["yT"]) for b in range(B)])
    return out.astype(np.float32)


# revision 31
# speedup vs baseline: 1.0032x; 1.0032x over previous
"""Trainium2 Bass kernel for a pre-norm transformer block (nn_Block_25752623907165).

Sharding: data-parallel over batch B=8 across the 8 NeuronCores (one batch
element per core, zero collectives).

v2 restructure vs baseline:
  - LN gains/biases folded into weights/biases on the host (qkv/fc1 weights
    absorb g; qkv/fc1 biases absorb b; the v-branch bias is folded into the
    proj bias since softmax rows sum to 1).
  - Attention inner loop software-pipelined: AV matmuls are emitted one unit
    behind their scores matmuls, with next-pair QK GEMM chunks (and later the
    invD-broadcast/normalize work) interleaved as PE filler, so the PE never
    idles on the ACT exp and the HAM clock gate stays at 2.4 GHz.
  - Chunked xT DMA + warm-up matmuls kill the cold start; proj is interleaved
    with LN2 stats; fc1's first m-group is pipelined against LN2 output chunks.
  - LayerNorm 2nd pass in bf16 (2x DVE rate); rstd = exp(-0.5*ln(var)) keeps
    ACT on the natural_log_exp table set through attention (2 table loads
    total); residual+bias fused in one scalar_tensor_tensor op.
"""

import numpy as np
import ml_dtypes
from collections import deque

EMBED = 1024
HEADS = 16
HEAD_DIM = 64
HIDDEN = 4096
N_TOK = 1024
B = 8
N_CORES = 8
EPS = 1e-5
P = 128
CSUB = EMBED // P          # 8
HSUB = HIDDEN // P         # 32
QW = 512                   # PSUM bank = 512 fp32

BF16 = ml_dtypes.bfloat16
FP8 = ml_dtypes.float8_e4m3
FP8_MAX = 224.0

_CACHE = {}


def _fp8_lhsT_chunks(w, n_mtiles):
    """[K, M] fp32 -> per-channel-scaled fp8 lhsT chunks + invscale percol.

    Returns (chunks [n_mtiles, 128, K//128, 128] fp8, invsc [128, n_mtiles] f32)
    """
    K, M = w.shape
    sc = FP8_MAX / np.abs(w).max(axis=0)          # [M]
    wq = np.clip(w * sc[None, :], -240., 240.).astype(FP8)
    a = wq.reshape(K // P, P, n_mtiles, P).transpose(2, 1, 0, 3)
    return np.ascontiguousarray(a), _pack_percol(1.0 / sc)


# ---------------------------------------------------------------------------
# host-side packing helpers
# ---------------------------------------------------------------------------

def _pack_lhsT_chunks(w, n_mtiles):
    """[K, M] fp32 -> [n_mtiles, 128, K//128, 128] bf16 (lhsT tiles for PE)."""
    K, M = w.shape
    a = w.reshape(K // P, P, n_mtiles, P).transpose(2, 1, 0, 3)
    return np.ascontiguousarray(a.astype(BF16))


def _pack_rhs(w):
    """[K, M] fp32 -> [128, K//128, M] bf16 (moving-operand layout)."""
    K, M = w.shape
    a = w.reshape(K // P, P, M).transpose(1, 0, 2)
    return np.ascontiguousarray(a.astype(BF16))


def _pack_percol(v):
    """[F] fp32 -> [128, F//128] fp32: column m holds features m*128..m*128+127."""
    F = v.shape[0]
    return np.ascontiguousarray(v.reshape(F // P, P).T.astype(np.float32))


def _pack_xT(xb):
    """[N, C] fp32 -> [128, C//128, N] fp32 (transposed, partition-major)."""
    xT = xb.T  # [C, N]
    a = xT.reshape(CSUB, P, N_TOK).transpose(1, 0, 2)
    return np.ascontiguousarray(a.astype(BF16))


def _unpack_yT(yT):
    """[128, C//128, N] fp32 -> [N, C] fp32."""
    full = yT.transpose(1, 0, 2).reshape(EMBED, N_TOK).astype(np.float32)
    return np.ascontiguousarray(full.T)


# ---------------------------------------------------------------------------
# kernel build
# ---------------------------------------------------------------------------

WV_SCALE = 64.0


def _build():
    import concourse.bacc as bacc
    import concourse.mybir as mybir
    import concourse.tile as tile
    from contextlib import ExitStack

    dt = mybir.dt
    AF = mybir.ActivationFunctionType
    OP = mybir.AluOpType

    nc = bacc.Bacc("TRN2", target_bir_lowering=False, debug=False)

    f32, bf16, fp8 = dt.float32, dt.bfloat16, dt.float8e4

    def dram(name, shape, d=f32, out=False):
        return nc.dram_tensor(name, list(shape), d,
                              kind="ExternalOutput" if out else "ExternalInput").ap()

    xT_d = dram("xT", [P, CSUB, N_TOK], bf16)
    wqk_d = dram("wqk", [16, P, CSUB, P], fp8)      # lhsT chunks, q|k features
    wv_d = dram("wv", [P, CSUB, EMBED], fp8)        # rhs layout
    bqk_d = dram("bqk", [P, 16])
    wpr_d = dram("wpr", [CSUB, P, CSUB, P], fp8)
    bpr_d = dram("bpr", [P, CSUB])
    wf1_d = dram("wf1", [HSUB, P, CSUB, P], fp8)
    bf1_d = dram("bf1", [P, HSUB])
    wf2_d = dram("wf2", [CSUB, P, HSUB, P], fp8)
    bf2_d = dram("bf2", [P, CSUB])
    iqk_d = dram("iqk", [P, 16])
    ipr_d = dram("ipr", [P, CSUB])
    if1_d = dram("if1", [P, HSUB])
    if2_d = dram("if2", [P, CSUB])
    yT_d = dram("yT", [P, CSUB, N_TOK], bf16, out=True)

    with tile.TileContext(nc) as tc, ExitStack() as ctx:
        const = ctx.enter_context(tc.tile_pool(name="const", bufs=1))
        persist = ctx.enter_context(tc.tile_pool(name="persist", bufs=1))
        smalls = ctx.enter_context(tc.tile_pool(name="smalls", bufs=1))
        w8 = ctx.enter_context(tc.tile_pool(name="w8", bufs=5))

        # ---- constants ---------------------------------------------------
        ones_sq = const.tile([P, P], bf16)      # 1/1024 : layernorm mean lhsT
        nc.vector.memset(ones_sq[:], 1.0 / EMBED)
        ones_bc = const.tile([P, P], bf16)      # 1.0 : K=1 broadcast lhsT rows
        nc.vector.memset(ones_bc[:], 1.0)
        warm_rhs = const.tile([P, QW], bf16)
        nc.vector.memset(warm_rhs[:], 0.001)
        ones_f8 = const.tile([P, 2, P], fp8)
        nc.vector.memset(ones_f8[:], 1.0 / EMBED)

        bqk_sb = const.tile([P, 16], f32)
        nc.sync.dma_start(bqk_sb[:], bqk_d[:])
        bpr_sb = const.tile([P, CSUB], f32)
        nc.sync.dma_start(bpr_sb[:], bpr_d[:])
        bf1_sb = const.tile([P, HSUB], f32)
        nc.sync.dma_start(bf1_sb[:], bf1_d[:])
        bf2_sb = const.tile([P, CSUB], f32)
        nc.sync.dma_start(bf2_sb[:], bf2_d[:])
        iqk_sb = const.tile([P, 16], f32)
        nc.sync.dma_start(iqk_sb[:], iqk_d[:])
        ipr_sb = const.tile([P, CSUB], f32)
        nc.sync.dma_start(ipr_sb[:], ipr_d[:])
        if1_sb = const.tile([P, HSUB], f32)
        nc.sync.dma_start(if1_sb[:], if1_d[:])
        if2_sb = const.tile([P, CSUB], f32)
        nc.sync.dma_start(if2_sb[:], if2_d[:])
        DR = mybir.MatmulPerfMode.DoubleRow

        # ---- persistent activations -------------------------------------
        xT = persist.tile([P, CSUB, N_TOK], bf16)

        # LN stat broadcast tiles (rotate bufs=1 between LN1/LN2)
        def ln_tiles(which):
            mu_bf = smalls.tile([P, N_TOK], bf16, tag="mu", name=f"mu{which}")
            rstd_bf = smalls.tile([P, N_TOK], bf16, tag="rstd",
                                  name=f"rstd{which}")
            mu2 = smalls.tile([P, N_TOK], bf16, tag="mu2", name=f"mu2{which}")
            var_t = smalls.tile([P, N_TOK], bf16, tag="var", name=f"var{which}")
            return mu_bf, rstd_bf, mu2, var_t

        def emit_ln_evict(mu_ps, sq_ps, mu_bf, rstd_bf, mu2, var_t):
            # mu broadcast (bf16), var = E[x^2]+EPS-mu^2, rstd = exp(-0.5*ln(var))
            nc.scalar.activation(mu_bf[:], mu_ps[:], AF.Copy)
            nc.vector.tensor_tensor(mu2[:], mu_bf[:], mu_bf[:], OP.mult)
            nc.vector.scalar_tensor_tensor(var_t[:], sq_ps[:], float(EPS),
                                           mu2[:], OP.add, OP.subtract)
            nc.scalar.activation(var_t[:], var_t[:], AF.Ln)
            nc.scalar.activation(rstd_bf[:], var_t[:], AF.Exp, scale=-0.5)

        # =================================================================
        # Phase 0+1: warmup + chunked x DMA + LN1 stats (pipelined)
        # x (bf16) is staged into hT, then layernormed in place.
        # =================================================================
        oT_cm = tc.tile_pool(name="oT_pool", bufs=1)
        oTp = oT_cm.__enter__()
        oT = oTp.tile([P, CSUB, N_TOK], fp8)

        attn_cm = tc.tile_pool(name="attn_sb", bufs=1)
        attn = attn_cm.__enter__()
        hT = attn.tile([P, CSUB, N_TOK], fp8)
        qkall = attn.tile([P, 16, N_TOK], bf16)
        v65 = attn.tile([P, CSUB, HEADS, 65], bf16)
        nc.vector.memset(v65[:, :, :, 64:65], 1.0)
        OSLOT = 9
        oU = attn.tile([64, OSLOT, N_TOK], bf16)
        DrA = attn.tile([8, N_TOK], f32)
        DrB = attn.tile([4, N_TOK], f32)
        DrC = attn.tile([4, N_TOK], f32)
        invA = attn.tile([8, N_TOK], f32, tag="inv", name="invA")
        invB = attn.tile([4, N_TOK], f32, tag="inv", name="invB")
        invC = attn.tile([4, N_TOK], f32, tag="inv", name="invC")
        invA_bf = attn.tile([8, N_TOK], bf16)
        invB_bf = attn.tile([4, N_TOK], bf16, tag="invBC_bf", name="invB_bf")
        invC_bf = attn.tile([4, N_TOK], bf16, tag="invBC_bf", name="invC_bf")

        wv_cm = tc.tile_pool(name="wv_pool", bufs=1)
        wvp = wv_cm.__enter__()
        wv_sb = wvp.tile([P, CSUB, EMBED], fp8)

        wqk_tiles = {}

        def fetch_wqk(m):
            wt = w8.tile([P, CSUB, P], fp8, tag="w8", name=f"wqk{m}")
            nc.sync.dma_start(wt[:], wqk_d[m])
            wqk_tiles[m] = wt

        ps_stat_cm = tc.tile_pool(name="ps_stat", bufs=1, space="PSUM")
        ps_stat = ps_stat_cm.__enter__()
        mu_ps = ps_stat.tile([P, N_TOK], f32, tag="mu_ps")
        sq_ps = ps_stat.tile([P, N_TOK], f32, tag="sq_ps")
        warm_ps = ps_stat.tile([P, N_TOK], f32, tag="warm")

        def warm(n):
            for _ in range(n):
                nc.tensor.matmul(warm_ps[:, 0:QW], ones_sq[:], warm_rhs[:])

        fetch_wqk(0); fetch_wqk(8)
        warm(10)
        for c in range(CSUB):
            nc.sync.dma_start(xT[:, c, :], xT_d[:, c, :])
            nc.sync.dma_start(wv_sb[:, c, :], wv_d[:, c, :])
            if c == 0:
                fetch_wqk(1); fetch_wqk(9)
            xbc = xT[:, c, :]
            sqc = smalls.tile([P, N_TOK], bf16, tag="sq", bufs=2,
                              name=f"sq{c}")
            nc.vector.tensor_tensor(sqc[:], xbc, xbc, OP.mult)
            for q in range(2):
                sl = slice(q * QW, (q + 1) * QW)
                nc.tensor.matmul(mu_ps[:, sl], ones_sq[:], xbc[:, sl],
                                 start=(c == 0), stop=(c == CSUB - 1))
                nc.tensor.matmul(sq_ps[:, sl], ones_sq[:], sqc[:, sl],
                                 start=(c == 0), stop=(c == CSUB - 1))
            warm(5)

        mu_bf, rstd_bf, mu2, var_t = ln_tiles(1)
        emit_ln_evict(mu_ps, sq_ps, mu_bf, rstd_bf, mu2, var_t)
        warm(26)

        # LN1 2nd pass: bf16 DVE subtract/mult, fp8 cast on ACT -> hT
        for c in range(CSUB):
            t = smalls.tile([P, N_TOK], bf16, tag="lnt", bufs=1,
                            name=f"lnt{c}")
            nc.vector.tensor_tensor(t[:], xT[:, c, :], mu_bf[:], OP.subtract)
            t2 = smalls.tile([P, N_TOK], bf16, tag="lnt2", bufs=2,
                             name=f"lnt2_{c}")
            nc.vector.tensor_tensor(t2[:], t[:], rstd_bf[:], OP.mult)
            nc.scalar.copy(hT[:, c, :], t2[:])

        ps_stat_cm.__exit__(None, None, None)

        # =================================================================
        # Phase 2: qkv GEMM groups (pair-0/1 qk + all v), c-major pipelined
        # =================================================================
        ps_g_cm = tc.tile_pool(name="ps_g", bufs=4, space="PSUM")
        ps_g = ps_g_cm.__enter__()

        def evict_qk(m, ps):
            nc.vector.tensor_scalar(qkall[:, m, :], ps[:],
                                    iqk_sb[:, m:m + 1], bqk_sb[:, m:m + 1],
                                    OP.mult, OP.add)

        def evict_v(mt, ps):
            nc.vector.tensor_scalar(
                v65[:, mt, :, 0:64],
                ps[:].rearrange("p (h d) -> p h d", d=64),
                1.0 / WV_SCALE, None, OP.mult)

        # group entries: ("qk", m) or ("v", mt)
        groups = [[("qk", 0), ("qk", 8), ("v", 0), ("v", 1)],
                  [("v", 2), ("v", 3), ("v", 4), ("v", 5)],
                  [("v", 6), ("v", 7)]]
        for gi, grp in enumerate(groups):
            pss = []
            for kind, m in grp:
                pss.append(ps_g.tile([P, N_TOK], f32, tag="g",
                                     name=f"g{gi}_{kind}{m}"))
            for c in range(0, CSUB, 2):
                for (kind, m), ps in zip(grp, pss):
                    for q in range(2):
                        sl = slice(q * QW, (q + 1) * QW)
                        if kind == "qk":
                            nc.tensor.matmul(ps[:, sl],
                                             wqk_tiles[m][:, c:c + 2, :],
                                             hT[:, c:c + 2, sl],
                                             start=(c == 0),
                                             stop=(c == CSUB - 2),
                                             perf_mode=DR)
                        else:
                            nc.tensor.matmul(ps[:, sl],
                                             hT[:, c:c + 2,
                                                m * P:(m + 1) * P],
                                             wv_sb[:, c:c + 2, sl],
                                             start=(c == 0),
                                             stop=(c == CSUB - 2),
                                             perf_mode=DR)
            for (kind, m), ps in zip(grp, pss):
                if kind == "qk":
                    evict_qk(m, ps)
                else:
                    evict_v(m, ps)

        ps_g_cm.__exit__(None, None, None)
        wv_cm.__exit__(None, None, None)

        # =================================================================
        # Phase 3: attention, software-pipelined with qk/normalize filler
        # =================================================================
        ps_at_cm = tc.tile_pool(name="ps_at", bufs=1, space="PSUM")
        ps_at = ps_at_cm.__enter__()
        # tags: s (scores, bufs=2), o (AV accum, bufs=1), q (filler, bufs=1)

        wpr_tiles = {}

        def fetch_wpr(m):
            wt = w8.tile([P, CSUB, P], fp8, tag="w8", name=f"wpr{m}")
            nc.sync.dma_start(wt[:], wpr_d[m])
            wpr_tiles[m] = wt

        filler = deque()

        def step(n=1):
            for _ in range(n):
                if filler:
                    filler.popleft()()

        def make_qk_filler_dr(m):
            """DoubleRow-rate qk filler (for pair-0 window: must finish fast)."""
            state = {}

            def s_cq(c, q):
                def run():
                    if c == 0 and q == 0:
                        state["ps"] = ps_at.tile([P, N_TOK], f32, tag="q",
                                                 bufs=1, name=f"qfd{m}")
                    sl = slice(q * QW, (q + 1) * QW)
                    nc.tensor.matmul(state["ps"][:, sl],
                                     wqk_tiles[m][:, c:c + 2, :],
                                     hT[:, c:c + 2, sl],
                                     start=(c == 0), stop=(c == CSUB - 2),
                                     perf_mode=DR)
                return run

            steps = [s_cq(c, q) for c in range(0, CSUB, 2) for q in range(2)]
            steps.append(lambda: evict_qk(m, state["ps"]))
            return steps

        def make_qk_filler(m):
            """Filler steps computing qk chunk m into qkall (psQ rotation)."""
            state = {}

            def s_cq(c, q):
                def run():
                    if c == 0 and q == 0:
                        state["ps"] = ps_at.tile([P, N_TOK], f32, tag="q",
                                                 bufs=1, name=f"qf{m}")
                    sl = slice(q * QW, (q + 1) * QW)
                    # deliberately NOT DoubleRow: filler density keeps the
                    # PE duty high enough that the HAM stays at K=8/8
                    nc.tensor.matmul(state["ps"][:, sl],
                                     wqk_tiles[m][:, c, :], hT[:, c, sl],
                                     start=(c == 0), stop=(c == CSUB - 1))
                return run

            steps = [s_cq(c, q) for c in range(CSUB) for q in range(2)]
            steps.append(lambda: evict_qk(m, state["ps"]))
            return steps

        iD_tiles = {}

        def load_iD_batch(bi, src, rows):
            iD = attn.tile([P, N_TOK], bf16, tag="iD64", bufs=2,
                           name=f"iDb{bi}")
            for j, r in enumerate(rows):
                nc.sync.dma_start(iD[32 * j:32 * j + 1, :], src[r:r + 1, :])
            iD_tiles[bi] = iD

        def make_junk_step(nm):
            def run():
                jt = ps_at.tile([P, N_TOK], f32, tag="s", bufs=2,
                                name=f"j{nm}")
                nc.tensor.matmul(jt[:, 0:QW], ones_sq[:], warm_rhs[:])
            return run

        def make_norm_filler(h):
            """Filler steps normalizing head h into oT (uses psQ for bc)."""
            bi, j = divmod(h, 4)
            state = {}

            def s_bc():
                bc = ps_at.tile([P, N_TOK], f32, tag="q", bufs=1,
                                name=f"bc{h}")
                r = 32 * j
                for q in range(2):
                    sl = slice(q * QW, (q + 1) * QW)
                    nc.tensor.matmul(bc[0:64, sl], ones_bc[r:r + 1, 0:64],
                                     iD_tiles[bi][r:r + 1, sl],
                                     tile_position=(r, 0))
                state["bc"] = bc

            def s_mul():
                if h % 2 == 0:
                    nc.vector.tensor_tensor(oT[0:64, h // 2, :],
                                            oU[:, h % OSLOT, :],
                                            state["bc"][0:64, :], OP.mult)
                else:
                    to = attn.tile([64, N_TOK], fp8, tag="to", bufs=1,
                                   name=f"to{h}")
                    nc.vector.tensor_tensor(to[:], oU[:, h % OSLOT, :],
                                            state["bc"][0:64, :], OP.mult)
                    nc.sync.dma_start(oT[64:128, h // 2, :], to[:])

            return [s_bc, s_mul]

        # attention unit machinery
        e_of = {}     # (h,k) -> exp tile
        o_ps_of = {}  # h -> AV psum tile

        def unit_scores(p, h, k):
            rows = slice(0, 64) if (h % 2) == 0 else slice(64, 128)
            s_ps = ps_at.tile([P, N_TOK], f32, tag="s", bufs=2,
                              name=f"s{h}_{k}")
            for q in range(2):
                sl = slice(q * QW, (q + 1) * QW)
                nc.tensor.matmul(s_ps[:, sl],
                                 qkall[rows, 8 + p, k * P:(k + 1) * P],
                                 qkall[rows, p, sl])
            e = attn.tile([P, N_TOK], bf16, tag="e", bufs=2, name=f"e{h}_{k}")
            nc.scalar.activation(e[:], s_ps[:], AF.Exp, scale=0.125)
            e_of[(h, k)] = e

        def unit_av(h, k):
            if k == 0:
                o_ps_of[h] = ps_at.tile([P, N_TOK], f32, tag="o", bufs=1,
                                        name=f"o{h}")
            o_ps = o_ps_of[h]
            e = e_of.pop((h, k))
            for q in range(2):
                sl = slice(q * QW, (q + 1) * QW)
                nc.tensor.matmul(o_ps[0:65, sl], v65[:, k, h, :], e[:, sl],
                                 start=(k == 0), stop=(k == CSUB - 1))

        def end_head(h):
            o_ps = o_ps_of.pop(h)
            nc.vector.tensor_copy(oU[:, h % OSLOT, :], o_ps[0:64, :])
            d64 = attn.tile([65, N_TOK], f32, tag="d64", bufs=1,
                            name=f"d64_{h}")
            nc.vector.tensor_copy(d64[64:65, :], o_ps[64:65, :])
            if h < 8:
                dst = DrA[h:h + 1, :]
            elif h < 12:
                dst = DrB[h - 8:h - 7, :]
            else:
                dst = DrC[h - 12:h - 11, :]
            nc.sync.dma_start(dst, d64[64:65, :])
            # denominator batches -> reciprocal + normalize fillers
            if h == 7:
                nc.vector.reciprocal_approx_fast(invA[:], DrA[:])
                nc.vector.tensor_scalar(invA_bf[:], invA[:], 16.0, None, OP.mult)
                load_iD_batch(0, invA_bf, [0, 1, 2, 3])
                load_iD_batch(1, invA_bf, [4, 5, 6, 7])
                for hh in range(0, 8):
                    filler.extend(make_norm_filler(hh))
            elif h == 11:
                nc.vector.reciprocal_approx_fast(invB[:], DrB[:])
                nc.vector.tensor_scalar(invB_bf[:], invB[:], 16.0, None, OP.mult)
                load_iD_batch(2, invB_bf, [0, 1, 2, 3])
                filler.append(make_junk_step("jb0"))
                filler.append(make_junk_step("jb1"))
                for hh in range(8, 12):
                    filler.extend(make_norm_filler(hh))
            elif h == 15:
                nc.vector.reciprocal_approx_fast(invC[:], DrC[:])
                nc.vector.tensor_scalar(invC_bf[:], invC[:], 16.0, None, OP.mult)
                load_iD_batch(3, invC_bf, [0, 1, 2, 3])
                for hh in range(12, 16):
                    filler.extend(make_norm_filler(hh))
                fetch_wpr(0)
                fetch_wpr(1)

        # run units with 1-unit AV lag; qk chunks for pair p+2 as filler
        pend = None
        for p in range(8):
            if p == 0:
                filler.extend(make_qk_filler_dr(1))
                filler.extend(make_qk_filler_dr(9))
            if p + 2 < 8:
                fetch_wqk(p + 2)
                fetch_wqk(8 + p + 2)
                filler.extend(make_qk_filler(p + 2))
                filler.extend(make_qk_filler(8 + p + 2))
            else:
                for jj in range(16):
                    filler.append(make_junk_step(f"p{p}_{jj}"))
            step(2)
            for h in (2 * p, 2 * p + 1):
                for k in range(CSUB):
                    unit_scores(p, h, k)
                    step()
                    if pend is not None:
                        unit_av(*pend)
                        if pend[1] == CSUB - 1:
                            end_head(pend[0])
                    pend = (h, k)
                    step()
        unit_av(*pend)
        end_head(pend[0])
        for jj in range(10):
            make_junk_step(f"dr{jj}")()
        ji = 0
        while filler:
            filler.popleft()()
            jt = ps_at.tile([P, N_TOK], f32, tag="s", bufs=2,
                            name=f"jnk{ji}")
            nc.tensor.matmul(jt[:, 0:QW], ones_sq[:], warm_rhs[:])
            ji += 1

        ps_at_cm.__exit__(None, None, None)
        attn_cm.__exit__(None, None, None)

        # =================================================================
        # Phase 4: proj + residual + LN2 stats (interleaved per m-chunk)
        # =================================================================
        mlp_cm = tc.tile_pool(name="mlp_sb", bufs=1)
        mlp = mlp_cm.__enter__()
        w32_cm = tc.tile_pool(name="w32", bufs=2)
        w32 = w32_cm.__enter__()
        ln2T = mlp.tile([P, CSUB, N_TOK], fp8)
        x2f8 = mlp.tile([P, CSUB, N_TOK], fp8)
        sq2f8 = mlp.tile([P, CSUB, N_TOK], fp8)
        geluT = mlp.tile([P, HSUB, N_TOK], fp8)

        ps_pr_cm = tc.tile_pool(name="ps_pr", bufs=1, space="PSUM")
        ps_pr = ps_pr_cm.__enter__()
        mu2_ps = ps_pr.tile([P, N_TOK], f32, tag="mu2_ps")
        sq2_ps = ps_pr.tile([P, N_TOK], f32, tag="sq2_ps")

        for m in range(CSUB):
            if m + 2 < CSUB:
                fetch_wpr(m + 2)
            p_ps = ps_pr.tile([P, N_TOK], f32, tag="p", bufs=2,
                              name=f"p{m}")
            for c in range(0, CSUB, 2):
                for q in range(2):
                    sl = slice(q * QW, (q + 1) * QW)
                    nc.tensor.matmul(p_ps[:, sl],
                                     wpr_tiles[m][:, c:c + 2, :],
                                     oT[:, c:c + 2, sl],
                                     start=(c == 0), stop=(c == CSUB - 2),
                                     perf_mode=DR)
            # unscale on ACT, then xT += t + bias (fused)
            t_pr = mlp.tile([P, N_TOK], bf16, tag="tpr", bufs=2,
                            name=f"tpr{m}")
            nc.scalar.activation(t_pr[:], p_ps[:], AF.Copy,
                                 scale=ipr_sb[:, m:m + 1])
            nc.vector.scalar_tensor_tensor(xT[:, m, :], t_pr[:],
                                           bpr_sb[:, m:m + 1], xT[:, m, :],
                                           OP.add, OP.add)
            # LN2 stats for this chunk (straight from bf16 xT)
            xbc = xT[:, m, :]
            sqc = mlp.tile([P, N_TOK], bf16, tag="sq2", bufs=2,
                           name=f"sqB{m}")
            nc.vector.tensor_tensor(sqc[:], xbc, xbc, OP.mult)
            for q in range(2):
                sl = slice(q * QW, (q + 1) * QW)
                nc.tensor.matmul(mu2_ps[:, sl], ones_sq[:], xbc[:, sl],
                                 start=(m == 0), stop=(m == CSUB - 1))
                nc.tensor.matmul(sq2_ps[:, sl], ones_sq[:], sqc[:, sl],
                                 start=(m == 0), stop=(m == CSUB - 1))

        mu_bf2, rstd_bf2, mu2b, var_t2 = ln_tiles(2)
        emit_ln_evict(mu2_ps, sq2_ps, mu_bf2, rstd_bf2, mu2b, var_t2)

        ps_pr_cm.__exit__(None, None, None)

        # =================================================================
        # Phase 5: LN2 2nd pass + fc1 (m-groups; G0 pipelined per chunk)
        # =================================================================
        ps_mm_cm = tc.tile_pool(name="ps_mm", bufs=4, space="PSUM")
        ps_mm = ps_mm_cm.__enter__()

        def warm_mm(n, nm):
            wps = ps_mm.tile([P, N_TOK], f32, tag="mm", name=f"wm{nm}")
            for _ in range(n):
                nc.tensor.matmul(wps[:, 0:QW], ones_sq[:], warm_rhs[:])

        wf1_tiles = {}

        def fetch_wf1(m):
            wt = w8.tile([P, CSUB, P], fp8, tag="w8", name=f"wf1_{m}")
            nc.sync.dma_start(wt[:], wf1_d[m])
            wf1_tiles[m] = wt

        for m in range(4):
            fetch_wf1(m)
        warm_mm(14, "a")

        # LN2 2nd pass chunks: bf16 DVE ops, fp8 cast on ACT -> ln2T
        for c in range(CSUB):
            t = mlp.tile([P, N_TOK], bf16, tag="lnt2", bufs=2,
                         name=f"lnB{c}")
            nc.vector.tensor_tensor(t[:], xT[:, c, :], mu_bf2[:],
                                    OP.subtract)
            t2 = mlp.tile([P, N_TOK], bf16, tag="lnt2b", bufs=2,
                          name=f"lnB2_{c}")
            nc.vector.tensor_tensor(t2[:], t[:], rstd_bf2[:], OP.mult)
            nc.scalar.copy(ln2T[:, c, :], t2[:])

        # G0: m-tiles 0..3, c-major (chases LN2 chunk production)
        g0 = [ps_mm.tile([P, N_TOK], f32, tag="mm", name=f"f1g0_{m}")
              for m in range(4)]
        for c in range(0, CSUB, 2):
            for m in range(4):
                for q in range(2):
                    sl = slice(q * QW, (q + 1) * QW)
                    nc.tensor.matmul(g0[m][:, sl],
                                     wf1_tiles[m][:, c:c + 2, :],
                                     ln2T[:, c:c + 2, sl],
                                     start=(c == 0), stop=(c == CSUB - 2),
                                     perf_mode=DR)
        for m in range(4):
            fetch_wf1(m + 4)
        for m in range(4):
            nc.scalar.activation(geluT[:, m, :], g0[m][:], AF.Gelu,
                                 bias=bf1_sb[:, m:m + 1],
                                 scale=if1_sb[:, m:m + 1])

        # G1..: m-major
        for m in range(4, HSUB):
            if m + 4 < HSUB:
                fetch_wf1(m + 4)
            f_ps = ps_mm.tile([P, N_TOK], f32, tag="mm", name=f"f1_{m}")
            for c in range(0, CSUB, 2):
                for q in range(2):
                    sl = slice(q * QW, (q + 1) * QW)
                    nc.tensor.matmul(f_ps[:, sl],
                                     wf1_tiles[m][:, c:c + 2, :],
                                     ln2T[:, c:c + 2, sl],
                                     start=(c == 0), stop=(c == CSUB - 2),
                                     perf_mode=DR)
            nc.scalar.activation(geluT[:, m, :], f_ps[:], AF.Gelu,
                                 bias=bf1_sb[:, m:m + 1],
                                 scale=if1_sb[:, m:m + 1])

        # =================================================================
        # Phase 6: fc2 + residual -> yT
        # =================================================================
        w2_tiles = {}

        def fetch_wf2(m):
            wt = w32.tile([P, HSUB, P], fp8, tag="w32", name=f"wf2_{m}")
            nc.sync.dma_start(wt[:], wf2_d[m])
            w2_tiles[m] = wt

        fetch_wf2(0)
        for m2 in range(CSUB):
            if m2 + 1 < CSUB:
                fetch_wf2(m2 + 1)
            y_ps = ps_mm.tile([P, N_TOK], f32, tag="mm", name=f"f2_{m2}")
            for k in range(0, HSUB, 2):
                for q in range(2):
                    sl = slice(q * QW, (q + 1) * QW)
                    nc.tensor.matmul(y_ps[:, sl],
                                     w2_tiles[m2][:, k:k + 2, :],
                                     geluT[:, k:k + 2, sl],
                                     start=(k == 0), stop=(k == HSUB - 2),
                                     perf_mode=DR)
            t_f2 = mlp.tile([P, N_TOK], bf16, tag="tpr", bufs=2,
                            name=f"tf2{m2}")
            nc.scalar.activation(t_f2[:], y_ps[:], AF.Copy,
                                 scale=if2_sb[:, m2:m2 + 1])
            nc.vector.scalar_tensor_tensor(xT[:, m2, :], t_f2[:],
                                           bf2_sb[:, m2:m2 + 1], xT[:, m2, :],
                                           OP.add, OP.add)
            nc.sync.dma_start(yT_d[:, m2, :], xT[:, m2, :])

        ps_mm_cm.__exit__(None, None, None)
        w32_cm.__exit__(None, None, None)
        mlp_cm.__exit__(None, None, None)
        oT_cm.__exit__(None, None, None)

    nc.compile()
    return nc


def get_nc():
    if "nc" not in _CACHE:
        _CACHE["nc"] = _build()
    return _CACHE["nc"]


def make_in_maps(x, qkv_w, qkv_b, proj_w, proj_b, fc1_w, fc1_b, fc2_w, fc2_b,
                 ln1_g, ln1_b, ln2_g, ln2_b):
    f = np.float32
    x = np.asarray(x, f)
    qkv_w = np.asarray(qkv_w, f); qkv_b = np.asarray(qkv_b, f)
    proj_w = np.asarray(proj_w, f); proj_b = np.asarray(proj_b, f)
    fc1_w = np.asarray(fc1_w, f); fc1_b = np.asarray(fc1_b, f)
    fc2_w = np.asarray(fc2_w, f); fc2_b = np.asarray(fc2_b, f)
    g1 = np.asarray(ln1_g, f); b1 = np.asarray(ln1_b, f)
    g2 = np.asarray(ln2_g, f); b2 = np.asarray(ln2_b, f)

    # fold LN1 gain/bias into qkv weights/bias
    W1 = qkv_w * g1[:, None]
    b1f = qkv_b + b1 @ qkv_w
    vb = b1f[2 * EMBED:]
    # v-branch bias folds into proj bias (softmax rows sum to 1)
    bprf = proj_b + vb @ proj_w
    # fold LN2 gain/bias into fc1
    W2 = fc1_w * g2[:, None]
    bf1f = fc1_b + b2 @ fc1_w

    wqk_q, iqk = _fp8_lhsT_chunks(W1[:, :2 * EMBED], 16)
    # v: per-tensor scale (uniform weights); proj absorbs the oT x16 scale
    wv_q = np.clip(W1[:, 2 * EMBED:] * WV_SCALE, -240., 240.).astype(FP8)
    Kv, Mv = wv_q.shape
    wv_q = np.ascontiguousarray(
        wv_q.reshape(Kv // P, P, Mv).transpose(1, 0, 2))
    wpr_q, ipr = _fp8_lhsT_chunks(proj_w / 16.0, CSUB)
    wf1_q, if1 = _fp8_lhsT_chunks(W2, HSUB)
    wf2_q, if2 = _fp8_lhsT_chunks(fc2_w, CSUB)
    shared = {
        "wqk": wqk_q, "iqk": iqk,
        "wv": wv_q,
        "bqk": _pack_percol(b1f[:2 * EMBED]),
        "wpr": wpr_q, "ipr": ipr,
        "bpr": _pack_percol(bprf),
        "wf1": wf1_q, "if1": if1,
        "bf1": _pack_percol(bf1f),
        "wf2": wf2_q, "if2": if2,
        "bf2": _pack_percol(fc2_b),
    }
    return [dict(shared, xT=_pack_xT(x[b])) for b in range(B)]


def kernel(**inputs):
    from concourse.bass_utils import run_bass_kernel_spmd

    nc = get_nc()
    in_maps = make_in_maps(**inputs)
    res = run_bass_kernel_spmd(nc, in_maps, core_ids=list(range(N_CORES)))
    out = np.stack([_unpack_yT(res.results[b]["yT"]) for b in range(B)])
    return out.astype(np.float32)
